# revision 18
# baseline (speedup 1.0000x reference)
"""MoE (top-2 of 8 experts) SwiGLU FFN on 8 Trainium2 NeuronCores.

Strategy (expert-parallel, per the sharding hint):
  - Router (x @ w_gate -> softmax -> top-2) computed host-side on jax-CPU with
    the exact ops the reference uses, so expert selection matches the
    reference bit-for-bit. This is the "dispatch tokens by topk_idx" step.
  - Core e receives only the tokens routed to expert e (gathered, transposed,
    and pre-cast to bf16 host-side), plus expert e's weights pre-packed into
    the SBUF tile layout. All cores run one SPMD program sized to
    cap = max tokens per expert (zero-padded).
  - Device computes y_e^T = wo_e^T @ (silu(wg_e^T x^T) * (wi_e^T x^T)) with
    bf16 matmuls accumulating in fp32 PSUM. Tokens stay on the PSUM free
    dimension throughout; lhsT operands are the natural wi/wg [C,H] and
    wo [H,C] layouts, so no on-device transposes are needed.
  - Host combines: out[t] = val0[t]*y_{e0}[t] + val1[t]*y_{e1}[t].

Single-dispatch perf structure (vs the previous 444.8us baseline):
  - One shared 8-bank PSUM pool (4 allocations per matmul group) so each
    group's banks come from two groups back: drains (silu/mul/copy) have a
    full group period to complete instead of stalling the next group's
    first matmul (~0.3-0.55us per group boundary before).
  - Stage-1 weights packed per (128-col pair) "unit" [128, CC*512] so one
    DMA feeds one matmul group row; unit 0 streamed in 8 cc-granular pieces
    interleaved with the first x tile so the PE starts ~2us in instead of
    waiting for the full 6.5MB input prologue.
  - Token tiles sized [292,292,320,512,512,256]: small first tiles for
    DMA-rate-limited startup, small last tile so the final PSUM evacuation
    tail is short.
  - Stage-2 PSUM evacuations alternate DVE / Act engines and the output
    DMAs ride the Act HWDGE queue, keeping the tail off the (weight-laden)
    SP queue.
"""

import numpy as np
import ml_dtypes

import concourse.bass as bass
import concourse.mybir as mybir
import concourse.tile as tile
from concourse.bass_utils import run_bass_kernel_spmd

N_CORES = 8
N_EXPERTS = 8
TOP_K = 2
B, T, C, H = 4, 2048, 1024, 2048
CC = C // 128            # contraction chunks over C (8)
HH = H // 128            # 128-col chunks over H (16)
UN = HH // 2             # stage-1 weight units, 2 h-chunks each (8)
UW = CC * 512            # unit width in elements (4096)
CBW = 512                # stage-2 weight block width (columns of C)
CB = C // CBW            # stage-2 blocks (2)
BLK2 = HH * CBW
BF16 = mybir.dt.bfloat16
F32 = mybir.dt.float32
ACT = mybir.ActivationFunctionType


def _split_multi_waits(nc, max_waits=1):
    """This walrus build rejects >1 sync-wait per instruction. Peel extra
    waits onto single-wait EventSemaphore instructions inserted just before,
    on the same engine (identical blocking semantics)."""
    n_split = 0
    for fn in nc.m.functions:
        for bb in fn.blocks:
            out = []
            changed = False
            for inst in bb.instructions:
                si = inst.sync_info
                waits = list(si.on_wait) if si is not None else []
                if len(waits) > max_waits:
                    head, keep = waits[:-max_waits], waits[-max_waits:]
                    for j, w in enumerate(head):
                        out.append(mybir.InstEventSemaphore(
                            name=f"{inst.name}-wspl{j}",
                            engine=inst.engine,
                            sync_info=mybir.SyncInfo(on_wait=[w], on_update=[]),
                        ))
                    inst.sync_info = mybir.SyncInfo(
                        on_wait=keep, on_update=list(si.on_update))
                    changed = True
                    n_split += 1
                out.append(inst)
            if changed:
                bb.instructions = out
    return n_split


TILE_MODE = "head_tail"   # "head_tail" | "even5"
MERGE_OUT = True          # merge each stage-2 group's 4 output DMAs into 1
BALANCE = True            # cap main segment at 2048 cols + overflow slot


def tok_tiles_for(cap):
    """Token tile widths: small leading tiles (DMA-rate-limited startup),
    512s in the middle, small final tile (short drain tail)."""
    if cap == 2184 and TILE_MODE == "even5":
        widths = [440, 440, 436, 436, 432]
    elif cap == 2184 and TILE_MODE.startswith("uniform"):
        w = int(TILE_MODE[len("uniform"):])
        assert cap % w == 0
        widths = [w] * (cap // w)
    elif cap == 2048:
        widths = [292, 292, 440, 512, 512]
    elif cap == 2184:
        widths = [292, 292, 320, 512, 512, 256]
    else:
        # generic fallback: first ~256, last ~256, 512s between, all mult-4
        widths = []
        rem = cap
        first = min(292, max(4, (cap // 8) & ~3))
        widths.append(first)
        rem -= first
        last = min(256, max(4, rem & ~3))
        mid = rem - last
        while mid > 512:
            w = min(512, mid - 4) if mid - 512 in (0, *range(4, 513)) else 512
            widths.append(w)
            mid -= w
        if mid:
            widths.append(mid)
        widths.append(last)
    assert sum(widths) == cap and all(w % 4 == 0 and w <= 512 for w in widths)
    tiles = []
    t0 = 0
    for w in widths:
        tiles.append((t0, w))
        t0 += w
    return tiles


def _s1_group(nc, ps_pool, sg_pool, w1t, xt, hT, cap, t0, tw, u, key):
    """One stage-1 matmul group: 4 chains (k x u/g) over one token tile,
    cycling 4 PSUM banks from the shared 8-buf pool."""
    ch = []
    for k in range(2):
        pu = ps_pool.tile([128, 512], F32, tag="ps",
                          name=f"pu_{key}_{u}_{k}")
        pg = ps_pool.tile([128, 512], F32, tag="ps",
                          name=f"pg_{key}_{u}_{k}")
        ch.append((k, pu, pg))
    for cc in range(CC):
        for k, pu, pg in ch:
            nc.tensor.matmul(
                pu[:, :tw],
                w1t[:, cc * 512 + k * 128:cc * 512 + (k + 1) * 128],
                xt[:, cc * tw:(cc + 1) * tw],
                start=(cc == 0), stop=(cc == CC - 1))
            nc.tensor.matmul(
                pg[:, :tw],
                w1t[:, cc * 512 + (2 + k) * 128:cc * 512 + (3 + k) * 128],
                xt[:, cc * tw:(cc + 1) * tw],
                start=(cc == 0), stop=(cc == CC - 1))
    for k, pu, pg in ch:
        hh = 2 * u + k
        sg = sg_pool.tile([128, 512], F32, tag="sg", name=f"sg_{key}_{u}_{k}")
        nc.scalar.activation(sg[:, :tw], pg[:, :tw], ACT.Silu)
        nc.vector.tensor_mul(hT[:, hh * cap + t0: hh * cap + t0 + tw],
                             pu[:, :tw], sg[:, :tw])


def _s2_group(nc, ps_pool, yo_pool, wob_t, hT, yt, cap, t0, tw, cb, key):
    """One stage-2 matmul group: 4 ci chains over one token tile, then
    PSUM evacuation split across DVE/Act and one merged output DMA."""
    pq = [ps_pool.tile([128, 512], F32, tag="ps", name=f"psq_{key}_{q}")
          for q in range(4)]
    for hh in range(HH):
        for ci in range(4):
            nc.tensor.matmul(
                pq[ci][:, :tw],
                wob_t[:, hh * CBW + ci * 128:hh * CBW + (ci + 1) * 128],
                hT[:, hh * cap + t0: hh * cap + t0 + tw],
                start=(hh == 0), stop=(hh == HH - 1))
    yo = yo_pool.tile([128, 4, 512], BF16, tag="yo", name=f"yo_{key}")
    for ci in range(4):
        if ci % 2 == 0:
            nc.vector.tensor_copy(yo[:, ci, :tw], pq[ci][:, :tw])
        else:
            nc.scalar.activation(yo[:, ci, :tw], pq[ci][:, :tw], ACT.Copy)
    # output DMAs ride the Act HWDGE queue, off the weight-laden SP queue
    c0 = cb * CBW
    if MERGE_OUT:
        nc.scalar.dma_start(
            yt[c0:c0 + CBW, t0:t0 + tw].rearrange("(c p) t -> p c t", p=128),
            yo[:, :, :tw])
    else:
        for ci in range(4):
            nc.scalar.dma_start(
                yt[c0 + ci * 128:c0 + (ci + 1) * 128, t0:t0 + tw],
                yo[:, ci, :tw])


def _emit_pass(nc, pools, tiles, cap, xtb, w1b, wob, yt, key, ov=None):
    """Emit the FFN (stage 1 + stage 2) over the given token tiles. When
    ov=(xob, w1o, wobo, yto, w_ov), an overflow token slot with its own
    weight stream is interleaved into the same unit/block loops so the
    duplicate weight DMA amortizes over the whole pass."""
    xb_pool, w1_pool, h_pool, w2_pool, sg_pool, yo_pool, ps_pool = pools

    # -- input DMA prologue; SP HWDGE queue is in-order, so call order here
    # is queue order: u0 piece0, x0, u0 rest, x1.. --
    w1t0 = w1_pool.tile([128, UW], BF16, tag="w1", name=f"w1t_{key}_0")
    nc.sync.dma_start(w1t0[:, 0:512], w1b[:, 0:512])
    xts = []
    off = 0
    for ti, (t0, tw) in enumerate(tiles):
        xt_t = xb_pool.tile([128, CC * tw], BF16, tag=f"xb{cap}_{ti}",
                            name=f"xt_{key}_{ti}")
        nc.sync.dma_start(xt_t[:], xtb[:, off:off + CC * tw])
        xts.append(xt_t)
        off += CC * tw
        if ti == 0:
            for cc in range(1, CC):
                nc.sync.dma_start(w1t0[:, cc * 512:(cc + 1) * 512],
                                  w1b[:, cc * 512:(cc + 1) * 512])
    if ov:
        xob, w1o, wobo, yto, w_ov = ov
        xot = xb_pool.tile([128, CC * w_ov], BF16, tag="xbov",
                           name=f"xot_{key}")
        nc.sync.dma_start(xot[:], xob[:])
        hTo = h_pool.tile([128, HH * w_ov], BF16, tag="hTov",
                          name=f"hTo_{key}")

    # hT = silu(x@wg) * (x@wi), transposed: [H, cap] bf16
    hT = h_pool.tile([128, HH * cap], BF16, tag=f"hT{cap}", name=f"hT_{key}")

    # ---- stage 1: per unit u (h-chunks 2u, 2u+1, wi+wg) ----
    for u in range(UN):
        if u == 0:
            w1t = w1t0
        else:
            w1t = w1_pool.tile([128, UW], BF16, tag="w1",
                               name=f"w1t_{key}_{u}")
            nc.sync.dma_start(w1t[:], w1b[:, u * UW:(u + 1) * UW])
        if ov:
            # overflow weights share the w1 tag rotation (bufs=3): the DMA
            # for unit u's overflow tile overlaps the main groups of u
            w1to = w1_pool.tile([128, UW], BF16, tag="w1",
                                name=f"w1to_{key}_{u}")
            nc.sync.dma_start(w1to[:], w1o[:, u * UW:(u + 1) * UW])
        for ti, (t0, tw) in enumerate(tiles):
            _s1_group(nc, ps_pool, sg_pool, w1t, xts[ti], hT, cap, t0, tw,
                      u, f"{key}_{ti}")
        if ov:
            _s1_group(nc, ps_pool, sg_pool, w1to, xot, hTo, w_ov, 0, w_ov,
                      u, f"{key}_ov")

    # ---- stage 2: yT = wo^T @ hT ----
    for cb in range(CB):
        wob_t = w2_pool.tile([128, BLK2], BF16, tag="wob",
                             name=f"wob_{key}_{cb}")
        nc.sync.dma_start(wob_t[:], wob[:, cb * BLK2:(cb + 1) * BLK2])
        if ov:
            wobo_t = w2_pool.tile([128, BLK2], BF16, tag="wobo", bufs=1,
                                  name=f"wobo_{key}_{cb}")
            nc.sync.dma_start(wobo_t[:],
                              wobo[:, cb * BLK2:(cb + 1) * BLK2])
        for ti, (t0, tw) in enumerate(tiles):
            _s2_group(nc, ps_pool, yo_pool, wob_t, hT, yt, cap, t0, tw,
                      cb, f"{key}_{cb}_{ti}")
        if ov:
            _s2_group(nc, ps_pool, yo_pool, wobo_t, hTo, yto, w_ov, 0,
                      w_ov, cb, f"{key}_{cb}_ov")


def build_program(cap, reps=1, w_ov=0):
    """One SPMD program: expert FFN over [cap] tokens (token dim = PSUM free
    dim everywhere), plus an optional overflow pass over [w_ov] tokens with
    its own weight inputs (load balancing: surplus tokens of overloaded
    experts run on other cores with a duplicated weight stream).
    reps>1 repeats the whole compute (timing only).

    DRAM inputs are already in SBUF tile layout, bf16:
      xtb [128, CC*cap]    token-tile-major per tok_tiles_for(cap):
                           xtb[p, off(ti)*CC + cc*tw + t] =
                           x^T[cc*128+p, t0(ti)+t]
      w1b [128, UN*UW]     w1b[p, (u*CC+cc)*512 + (path*2+k)*128 + m] =
                           W_path[cc*128+p, (2u+k)*128+m], path 0=wi, 1=wg
      wob [128, CB*BLK2]   wob[p, (cb*HH+hh)*CBW+f] = wo[hh*128+p, cb*CBW+f]
      (+ xob/w1o/wobo, same layouts, when w_ov > 0)
    Output yt [C, cap] bf16 (y^T, one row block per c-chunk), + yto.
    """
    tiles = tok_tiles_for(cap)

    nc = bass.Bass()
    xtb = nc.dram_tensor("xtb", [128, CC * cap], BF16, kind="ExternalInput")
    w1b = nc.dram_tensor("w1b", [128, UN * UW], BF16, kind="ExternalInput")
    wob = nc.dram_tensor("wob", [128, CB * BLK2], BF16, kind="ExternalInput")
    yt = nc.dram_tensor("yt", [C, cap], BF16, kind="ExternalOutput")
    if w_ov:
        xob = nc.dram_tensor("xob", [128, CC * w_ov], BF16,
                             kind="ExternalInput")
        w1o = nc.dram_tensor("w1o", [128, UN * UW], BF16,
                             kind="ExternalInput")
        wobo = nc.dram_tensor("wobo", [128, CB * BLK2], BF16,
                              kind="ExternalInput")
        yto = nc.dram_tensor("yto", [C, w_ov], BF16, kind="ExternalOutput")

    with tile.TileContext(nc) as tc:
        with tc.tile_pool(name="xb", bufs=1) as xb_pool, \
             tc.tile_pool(name="w1", bufs=3 if w_ov else 2) as w1_pool, \
             tc.tile_pool(name="hT", bufs=1) as h_pool, \
             tc.tile_pool(name="w2", bufs=2) as w2_pool, \
             tc.tile_pool(name="sg", bufs=2 if w_ov else 3) as sg_pool, \
             tc.tile_pool(name="yo", bufs=2) as yo_pool, \
             tc.tile_pool(name="ps", bufs=8, space="PSUM") as ps_pool:
            pools = (xb_pool, w1_pool, h_pool, w2_pool, sg_pool, yo_pool,
                     ps_pool)
            for _rep in range(reps):
                ov = (xob, w1o, wobo, yto, w_ov) if w_ov else None
                _emit_pass(nc, pools, tiles, cap, xtb, w1b, wob, yt,
                           key=f"{_rep}m", ov=ov)
    _split_multi_waits(nc)
    return nc


def pack_w1(wi_e, wg_e):
    """wi/wg [C, H] f32 -> [128, UN*UW] bf16 in the w1b DRAM layout."""
    a = np.asarray(wi_e).reshape(CC, 128, UN, 2, 128)   # [cc, p, u, k, m]
    b = np.asarray(wg_e).reshape(CC, 128, UN, 2, 128)
    s = np.stack([a, b], axis=3)                        # [cc, p, u, path, k, m]
    s = s.transpose(1, 2, 0, 3, 4, 5)                   # [p, u, cc, path, k, m]
    return np.ascontiguousarray(s.reshape(128, UN * UW)
                                ).astype(ml_dtypes.bfloat16)


def pack_wo(w):
    """wo [H, C] f32 -> [128, CB*BLK2] bf16 in the wob DRAM layout."""
    a = np.asarray(w).reshape(HH, 128, CB, CBW)          # [hh, p, cb, f]
    a = a.transpose(1, 2, 0, 3)                          # [p, cb, hh, f]
    return np.ascontiguousarray(a.reshape(128, CB * BLK2)
                                ).astype(ml_dtypes.bfloat16)


def pack_x(x_disp_T, tiles):
    """x^T dispatch slab [C, cap] f32 -> [128, CC*cap] bf16, tile-major
    per the given (t0, tw) tile list."""
    a = x_disp_T.reshape(CC, 128, x_disp_T.shape[1])        # [cc, p, t]
    parts = []
    for t0, tw in tiles:
        blk = a[:, :, t0:t0 + tw].transpose(1, 0, 2)        # [p, cc, tw]
        parts.append(blk.reshape(128, CC * tw))
    return np.ascontiguousarray(np.concatenate(parts, axis=1)
                                ).astype(ml_dtypes.bfloat16)


def _route(x, w_gate):
    """Host-side router. Runs the exact reference ops on jax-CPU so the
    top-2 selection and gate values match the reference bit-for-bit."""
    import jax
    import jax.numpy as jnp
    cpu = jax.devices("cpu")[0]
    with jax.default_device(cpu):
        xj = jnp.asarray(np.asarray(x))
        wj = jnp.asarray(np.asarray(w_gate))
        logits = jnp.einsum("btc,ce->bte", xj, wj)
        gates = jax.nn.softmax(logits, axis=-1)
        topk_vals, topk_idx = jax.lax.top_k(gates, TOP_K)
    return (np.asarray(topk_vals).reshape(-1, TOP_K),
            np.asarray(topk_idx).reshape(-1, TOP_K))


OV_W = 0          # overflow slot width of the last make_in_maps call
OV_PLAN = None    # per-core (src_expert, start_in_expert, count)


def _plan_overflow(loads, capm):
    """Split each expert's surplus beyond capm into chunks of one width
    W that fit the 8 per-core overflow slots. Returns (W, plan) where
    plan[c] = (src_expert, start_in_expert, count)."""
    ovf = [(e, n - capm) for e, n in enumerate(loads) if n > capm]
    if not ovf:
        return 0, [(c, 0, 0) for c in range(N_CORES)]
    W = 0
    for Wc in range(4, 4096, 4):
        if sum(-(-o // Wc) for _, o in ovf) <= N_CORES:
            W = Wc
            break
    assert W, f"overflow does not fit: {loads}"
    plan = []
    for e, o in ovf:
        s = 0
        while s < o:
            cnt = min(W, o - s)
            plan.append((e, capm + s, cnt))
            s += cnt
    while len(plan) < N_CORES:
        plan.append((len(plan) % N_EXPERTS, 0, 0))
    return W, plan


def _dispatch(x, topk_idx):
    """Token lists per expert, (token, slot) positions, cap, and the
    gathered+packed per-expert xtb slabs (+ overflow slabs when balancing)."""
    global OV_W, OV_PLAN
    N = x.shape[0] * x.shape[1] if x.ndim == 3 else x.shape[0]
    x_flat = np.ascontiguousarray(np.asarray(x).reshape(N, C))
    idx_lists = []
    pos = np.empty((N, TOP_K), dtype=np.int64)
    for e in range(N_EXPERTS):
        sel = (topk_idx == e)
        toks = np.flatnonzero(sel.any(axis=1))
        idx_lists.append(toks)
        pos_of = np.full(N, -1, dtype=np.int64)
        pos_of[toks] = np.arange(len(toks))
        for k in range(TOP_K):
            m = sel[:, k]
            pos[m, k] = pos_of[m]
    loads = [len(t) for t in idx_lists]
    max_cnt = max(loads)
    if BALANCE and max_cnt > 2048:
        # pick (cap_main, W) minimizing total columns cap_main + W
        best = None
        for capm in range(2048, 1916, -4):
            W, _ = _plan_overflow(loads, capm)
            if best is None or capm + W < best[0]:
                best = (capm + W, capm)
        cap = best[1]
        OV_W, OV_PLAN = _plan_overflow(loads, cap)
    else:
        cap = max(512, -(-max_cnt // 4) * 4)
        OV_W, OV_PLAN = 0, None
    tiles = tok_tiles_for(cap)

    xT = np.ascontiguousarray(x_flat.T)            # [C, N]
    xtbs, xobs = [], []
    for e in range(N_EXPERTS):
        toks = idx_lists[e]
        slab = np.zeros((C, cap), dtype=np.float32)
        n_main = min(len(toks), cap)
        slab[:, :n_main] = xT[:, toks[:n_main]]
        xtbs.append(pack_x(slab, tiles))
    if OV_W:
        for c in range(N_CORES):
            se, start, cnt = OV_PLAN[c]
            slab = np.zeros((C, OV_W), dtype=np.float32)
            if cnt:
                slab[:, :cnt] = xT[:, idx_lists[se][start:start + cnt]]
            xobs.append(pack_x(slab, [(0, OV_W)]))
    return idx_lists, pos, cap, (xtbs, xobs)


def make_in_maps(x, wi, wg, wo, topk_idx):
    idx_lists, pos, cap, (xtbs, xobs) = _dispatch(x, topk_idx)
    w1_packed = [None] * N_EXPERTS
    wo_packed = [None] * N_EXPERTS

    def w1p(e):
        if w1_packed[e] is None:
            w1_packed[e] = pack_w1(wi[e], wg[e])
        return w1_packed[e]

    def wop(e):
        if wo_packed[e] is None:
            wo_packed[e] = pack_wo(wo[e])
        return wo_packed[e]

    in_maps = []
    for e in range(N_EXPERTS):
        m = {"xtb": xtbs[e], "w1b": w1p(e), "wob": wop(e)}
        if OV_W:
            se = OV_PLAN[e][0]
            m["xob"] = xobs[e]
            m["w1o"] = w1p(se)
            m["wobo"] = wop(se)
        in_maps.append(m)
    return idx_lists, pos, cap, in_maps


def kernel(x, w_gate, wi, wg, wo):
    x = np.asarray(x)
    wi, wg, wo = np.asarray(wi), np.asarray(wg), np.asarray(wo)
    N = B * T

    topk_vals, topk_idx = _route(x, w_gate)
    idx_lists, pos, cap, in_maps = make_in_maps(x, wi, wg, wo, topk_idx)

    nc = build_program(cap, w_ov=OV_W)
    res = run_bass_kernel_spmd(nc, in_maps, core_ids=list(range(N_CORES)))

    # combine: out[t] = sum_k vals[t,k] * y_{idx[t,k]}[t]
    cap_full = max(cap, max(len(t) for t in idx_lists))
    Y = np.zeros((N_EXPERTS, cap_full, C), dtype=np.float32)  # token-major
    for e in range(N_EXPERTS):
        Y[e, :cap] = res.results[e]["yt"].astype(np.float32).T
    if OV_W:
        for c in range(N_CORES):
            se, start, cnt = OV_PLAN[c]
            if cnt:
                Y[se, start:start + cnt] = \
                    res.results[c]["yto"].astype(np.float32).T[:cnt]
    out = (topk_vals[:, 0:1] * Y[topk_idx[:, 0], pos[:, 0], :]
           + topk_vals[:, 1:2] * Y[topk_idx[:, 1], pos[:, 1], :])
    return out.reshape(B, T, C).astype(np.float32)


# revision 19
# speedup vs baseline: 13.0903x; 13.0903x over previous
"""MoE (top-2 of 8 experts) SwiGLU FFN on 8 Trainium2 NeuronCores.

Strategy (expert-parallel, per the sharding hint):
  - Router (x @ w_gate -> softmax -> top-2) computed host-side on jax-CPU with
    the exact ops the reference uses, so expert selection matches the
    reference bit-for-bit. This is the "dispatch tokens by topk_idx" step.
  - Core e receives only the tokens routed to expert e (gathered, transposed,
    and pre-cast to bf16 host-side), plus expert e's weights pre-packed into
    the SBUF tile layout. All cores run one SPMD program sized to
    cap = max tokens per expert (zero-padded).
  - Device computes y_e^T = wo_e^T @ (silu(wg_e^T x^T) * (wi_e^T x^T)) with
    bf16 matmuls accumulating in fp32 PSUM. Tokens stay on the PSUM free
    dimension throughout; lhsT operands are the natural wi/wg [C,H] and
    wo [H,C] layouts, so no on-device transposes are needed.
  - Host combines: out[t] = val0[t]*y_{e0}[t] + val1[t]*y_{e1}[t].

Single-dispatch perf structure (vs the previous 444.8us baseline):
  - Load balance: expert loads are 1932..2182 (mean 2048), so the main
    segment is capped at cap_main (~2028) columns and each core carries one
    fixed-width overflow slot (~56 cols) holding another expert's surplus
    tokens, with that expert's weights streamed as extra inputs. The
    overflow groups are interleaved into the main unit/block loops so the
    duplicate weight stream amortizes over the whole pass (~67GB/s).
    Total columns per core: 2084 vs 2184 unbalanced.
  - One shared 8-bank PSUM pool (4 allocations per matmul group) so each
    group's banks come from two groups back: drains (silu/mul/copy) have a
    full group period to complete instead of stalling the next group's
    first matmul (~0.3-0.55us per group boundary before).
  - Stage-1 weights packed per (128-col pair) "unit" [128, CC*512] so one
    DMA feeds one matmul group row; unit 0 streamed in 8 cc-granular pieces
    interleaved with the first x tile so the PE starts ~2us in instead of
    waiting for the full 6.5MB input prologue (SP HWDGE queue is in-order,
    so dma_start call order is queue order).
  - Token tiles: small first tiles (DMA-rate-limited startup), small last
    tile (short final PSUM-evacuation tail).
  - Stage-2 PSUM evacuations alternate DVE / Act engines, are gathered in a
    [128,4,512] tile, and leave in ONE merged DMA per group on the Act
    HWDGE queue, keeping the tail off the (weight-laden) SP queue.
Measured (slope-timed, chained-donation dispatches): single-dispatch
~461-466us vs ~500-516us for the unbalanced layout and the 444.8us graded
baseline; TimelineSim predicts 350.0us vs 405.5us for the baseline.
"""

import numpy as np
import ml_dtypes

import concourse.bass as bass
import concourse.mybir as mybir
import concourse.tile as tile
from concourse.bass_utils import run_bass_kernel_spmd

N_CORES = 8
N_EXPERTS = 8
TOP_K = 2
B, T, C, H = 4, 2048, 1024, 2048
CC = C // 128            # contraction chunks over C (8)
HH = H // 128            # 128-col chunks over H (16)
UN = HH // 2             # stage-1 weight units, 2 h-chunks each (8)
UW = CC * 512            # unit width in elements (4096)
CBW = 512                # stage-2 weight block width (columns of C)
CB = C // CBW            # stage-2 blocks (2)
BLK2 = HH * CBW
BF16 = mybir.dt.bfloat16
F32 = mybir.dt.float32
ACT = mybir.ActivationFunctionType


def _split_multi_waits(nc, max_waits=1):
    """This walrus build rejects >1 sync-wait per instruction. Peel extra
    waits onto single-wait EventSemaphore instructions inserted just before,
    on the same engine (identical blocking semantics)."""
    n_split = 0
    for fn in nc.m.functions:
        for bb in fn.blocks:
            out = []
            changed = False
            for inst in bb.instructions:
                si = inst.sync_info
                waits = list(si.on_wait) if si is not None else []
                if len(waits) > max_waits:
                    head, keep = waits[:-max_waits], waits[-max_waits:]
                    for j, w in enumerate(head):
                        out.append(mybir.InstEventSemaphore(
                            name=f"{inst.name}-wspl{j}",
                            engine=inst.engine,
                            sync_info=mybir.SyncInfo(on_wait=[w], on_update=[]),
                        ))
                    inst.sync_info = mybir.SyncInfo(
                        on_wait=keep, on_update=list(si.on_update))
                    changed = True
                    n_split += 1
                out.append(inst)
            if changed:
                bb.instructions = out
    return n_split


TILE_MODE = "head_tail"   # "head_tail" | "even5"
MERGE_OUT = True          # merge each stage-2 group's 4 output DMAs into 1
BALANCE = True            # cap main segment at 2048 cols + overflow slot


def tok_tiles_for(cap):
    """Token tile widths: small leading tiles (DMA-rate-limited startup),
    512s in the middle, small final tile (short drain tail)."""
    if cap == 2184 and TILE_MODE == "even5":
        widths = [440, 440, 436, 436, 432]
    elif cap == 2184 and TILE_MODE.startswith("uniform"):
        w = int(TILE_MODE[len("uniform"):])
        assert cap % w == 0
        widths = [w] * (cap // w)
    elif cap == 2048:
        widths = [292, 292, 440, 512, 512]
    elif cap == 2184:
        widths = [292, 292, 320, 512, 512, 256]
    else:
        # generic fallback: first ~256, last ~256, 512s between, all mult-4
        widths = []
        rem = cap
        first = min(292, max(4, (cap // 8) & ~3))
        widths.append(first)
        rem -= first
        last = min(256, max(4, rem & ~3))
        mid = rem - last
        while mid > 512:
            w = min(512, mid - 4) if mid - 512 in (0, *range(4, 513)) else 512
            widths.append(w)
            mid -= w
        if mid:
            widths.append(mid)
        widths.append(last)
    assert sum(widths) == cap and all(w % 4 == 0 and w <= 512 for w in widths)
    tiles = []
    t0 = 0
    for w in widths:
        tiles.append((t0, w))
        t0 += w
    return tiles


def _s1_group(nc, ps_pool, sg_pool, w1t, xt, hT, cap, t0, tw, u, key):
    """One stage-1 matmul group: 4 chains (k x u/g) over one token tile,
    cycling 4 PSUM banks from the shared 8-buf pool."""
    ch = []
    for k in range(2):
        pu = ps_pool.tile([128, 512], F32, tag="ps",
                          name=f"pu_{key}_{u}_{k}")
        pg = ps_pool.tile([128, 512], F32, tag="ps",
                          name=f"pg_{key}_{u}_{k}")
        ch.append((k, pu, pg))
    for cc in range(CC):
        for k, pu, pg in ch:
            nc.tensor.matmul(
                pu[:, :tw],
                w1t[:, cc * 512 + k * 128:cc * 512 + (k + 1) * 128],
                xt[:, cc * tw:(cc + 1) * tw],
                start=(cc == 0), stop=(cc == CC - 1))
            nc.tensor.matmul(
                pg[:, :tw],
                w1t[:, cc * 512 + (2 + k) * 128:cc * 512 + (3 + k) * 128],
                xt[:, cc * tw:(cc + 1) * tw],
                start=(cc == 0), stop=(cc == CC - 1))
    for k, pu, pg in ch:
        hh = 2 * u + k
        sg = sg_pool.tile([128, 512], F32, tag="sg", name=f"sg_{key}_{u}_{k}")
        nc.scalar.activation(sg[:, :tw], pg[:, :tw], ACT.Silu)
        nc.vector.tensor_mul(hT[:, hh * cap + t0: hh * cap + t0 + tw],
                             pu[:, :tw], sg[:, :tw])


def _s2_group(nc, ps_pool, yo_pool, wob_t, hT, yt, cap, t0, tw, cb, key):
    """One stage-2 matmul group: 4 ci chains over one token tile, then
    PSUM evacuation split across DVE/Act and one merged output DMA."""
    pq = [ps_pool.tile([128, 512], F32, tag="ps", name=f"psq_{key}_{q}")
          for q in range(4)]
    for hh in range(HH):
        for ci in range(4):
            nc.tensor.matmul(
                pq[ci][:, :tw],
                wob_t[:, hh * CBW + ci * 128:hh * CBW + (ci + 1) * 128],
                hT[:, hh * cap + t0: hh * cap + t0 + tw],
                start=(hh == 0), stop=(hh == HH - 1))
    yo = yo_pool.tile([128, 4, 512], BF16, tag="yo", name=f"yo_{key}")
    for ci in range(4):
        if ci % 2 == 0:
            nc.vector.tensor_copy(yo[:, ci, :tw], pq[ci][:, :tw])
        else:
            nc.scalar.activation(yo[:, ci, :tw], pq[ci][:, :tw], ACT.Copy)
    # output DMAs ride the Act HWDGE queue, off the weight-laden SP queue
    c0 = cb * CBW
    if MERGE_OUT:
        nc.scalar.dma_start(
            yt[c0:c0 + CBW, t0:t0 + tw].rearrange("(c p) t -> p c t", p=128),
            yo[:, :, :tw])
    else:
        for ci in range(4):
            nc.scalar.dma_start(
                yt[c0 + ci * 128:c0 + (ci + 1) * 128, t0:t0 + tw],
                yo[:, ci, :tw])


def _emit_pass(nc, pools, tiles, cap, xtb, w1b, wob, yt, key, ov=None):
    """Emit the FFN (stage 1 + stage 2) over the given token tiles. When
    ov=(xob, w1o, wobo, yto, w_ov), an overflow token slot with its own
    weight stream is interleaved into the same unit/block loops so the
    duplicate weight DMA amortizes over the whole pass."""
    xb_pool, w1_pool, h_pool, w2_pool, sg_pool, yo_pool, ps_pool = pools

    # -- input DMA prologue; SP HWDGE queue is in-order, so call order here
    # is queue order: u0 piece0, x0, u0 rest, x1.. --
    w1t0 = w1_pool.tile([128, UW], BF16, tag="w1", name=f"w1t_{key}_0")
    nc.sync.dma_start(w1t0[:, 0:512], w1b[:, 0:512])
    xts = []
    off = 0
    for ti, (t0, tw) in enumerate(tiles):
        xt_t = xb_pool.tile([128, CC * tw], BF16, tag=f"xb{cap}_{ti}",
                            name=f"xt_{key}_{ti}")
        nc.sync.dma_start(xt_t[:], xtb[:, off:off + CC * tw])
        xts.append(xt_t)
        off += CC * tw
        if ti == 0:
            for cc in range(1, CC):
                nc.sync.dma_start(w1t0[:, cc * 512:(cc + 1) * 512],
                                  w1b[:, cc * 512:(cc + 1) * 512])
    if ov:
        xob, w1o, wobo, yto, w_ov = ov
        xot = xb_pool.tile([128, CC * w_ov], BF16, tag="xbov",
                           name=f"xot_{key}")
        nc.sync.dma_start(xot[:], xob[:])
        hTo = h_pool.tile([128, HH * w_ov], BF16, tag="hTov",
                          name=f"hTo_{key}")

    # hT = silu(x@wg) * (x@wi), transposed: [H, cap] bf16
    hT = h_pool.tile([128, HH * cap], BF16, tag=f"hT{cap}", name=f"hT_{key}")

    # ---- stage 1: per unit u (h-chunks 2u, 2u+1, wi+wg) ----
    for u in range(UN):
        if u == 0:
            w1t = w1t0
        else:
            w1t = w1_pool.tile([128, UW], BF16, tag="w1",
                               name=f"w1t_{key}_{u}")
            nc.sync.dma_start(w1t[:], w1b[:, u * UW:(u + 1) * UW])
        if ov:
            # overflow weights share the w1 tag rotation (bufs=3): the DMA
            # for unit u's overflow tile overlaps the main groups of u
            w1to = w1_pool.tile([128, UW], BF16, tag="w1",
                                name=f"w1to_{key}_{u}")
            nc.sync.dma_start(w1to[:], w1o[:, u * UW:(u + 1) * UW])
        for ti, (t0, tw) in enumerate(tiles):
            _s1_group(nc, ps_pool, sg_pool, w1t, xts[ti], hT, cap, t0, tw,
                      u, f"{key}_{ti}")
        if ov:
            _s1_group(nc, ps_pool, sg_pool, w1to, xot, hTo, w_ov, 0, w_ov,
                      u, f"{key}_ov")

    # ---- stage 2: yT = wo^T @ hT ----
    for cb in range(CB):
        wob_t = w2_pool.tile([128, BLK2], BF16, tag="wob",
                             name=f"wob_{key}_{cb}")
        nc.sync.dma_start(wob_t[:], wob[:, cb * BLK2:(cb + 1) * BLK2])
        if ov:
            wobo_t = w2_pool.tile([128, BLK2], BF16, tag="wobo", bufs=1,
                                  name=f"wobo_{key}_{cb}")
            nc.sync.dma_start(wobo_t[:],
                              wobo[:, cb * BLK2:(cb + 1) * BLK2])
        for ti, (t0, tw) in enumerate(tiles):
            _s2_group(nc, ps_pool, yo_pool, wob_t, hT, yt, cap, t0, tw,
                      cb, f"{key}_{cb}_{ti}")
        if ov:
            _s2_group(nc, ps_pool, yo_pool, wobo_t, hTo, yto, w_ov, 0,
                      w_ov, cb, f"{key}_{cb}_ov")


def build_program(cap, reps=1, w_ov=0):
    """One SPMD program: expert FFN over [cap] tokens (token dim = PSUM free
    dim everywhere), plus an optional overflow pass over [w_ov] tokens with
    its own weight inputs (load balancing: surplus tokens of overloaded
    experts run on other cores with a duplicated weight stream).
    reps>1 repeats the whole compute (timing only).

    DRAM inputs are already in SBUF tile layout, bf16:
      xtb [128, CC*cap]    token-tile-major per tok_tiles_for(cap):
                           xtb[p, off(ti)*CC + cc*tw + t] =
                           x^T[cc*128+p, t0(ti)+t]
      w1b [128, UN*UW]     w1b[p, (u*CC+cc)*512 + (path*2+k)*128 + m] =
                           W_path[cc*128+p, (2u+k)*128+m], path 0=wi, 1=wg
      wob [128, CB*BLK2]   wob[p, (cb*HH+hh)*CBW+f] = wo[hh*128+p, cb*CBW+f]
      (+ xob/w1o/wobo, same layouts, when w_ov > 0)
    Output yt [C, cap] bf16 (y^T, one row block per c-chunk), + yto.
    """
    tiles = tok_tiles_for(cap)

    nc = bass.Bass()
    xtb = nc.dram_tensor("xtb", [128, CC * cap], BF16, kind="ExternalInput")
    w1b = nc.dram_tensor("w1b", [128, UN * UW], BF16, kind="ExternalInput")
    wob = nc.dram_tensor("wob", [128, CB * BLK2], BF16, kind="ExternalInput")
    yt = nc.dram_tensor("yt", [C, cap], BF16, kind="ExternalOutput")
    if w_ov:
        xob = nc.dram_tensor("xob", [128, CC * w_ov], BF16,
                             kind="ExternalInput")
        w1o = nc.dram_tensor("w1o", [128, UN * UW], BF16,
                             kind="ExternalInput")
        wobo = nc.dram_tensor("wobo", [128, CB * BLK2], BF16,
                              kind="ExternalInput")
        yto = nc.dram_tensor("yto", [C, w_ov], BF16, kind="ExternalOutput")

    with tile.TileContext(nc) as tc:
        with tc.tile_pool(name="xb", bufs=1) as xb_pool, \
             tc.tile_pool(name="w1", bufs=3 if w_ov else 2) as w1_pool, \
             tc.tile_pool(name="hT", bufs=1) as h_pool, \
             tc.tile_pool(name="w2", bufs=2) as w2_pool, \
             tc.tile_pool(name="sg", bufs=2 if w_ov else 3) as sg_pool, \
             tc.tile_pool(name="yo", bufs=2) as yo_pool, \
             tc.tile_pool(name="ps", bufs=8, space="PSUM") as ps_pool:
            pools = (xb_pool, w1_pool, h_pool, w2_pool, sg_pool, yo_pool,
                     ps_pool)
            for _rep in range(reps):
                ov = (xob, w1o, wobo, yto, w_ov) if w_ov else None
                _emit_pass(nc, pools, tiles, cap, xtb, w1b, wob, yt,
                           key=f"{_rep}m", ov=ov)
    _split_multi_waits(nc)
    return nc


def pack_w1(wi_e, wg_e):
    """wi/wg [C, H] f32 -> [128, UN*UW] bf16 in the w1b DRAM layout."""
    a = np.asarray(wi_e).reshape(CC, 128, UN, 2, 128)   # [cc, p, u, k, m]
    b = np.asarray(wg_e).reshape(CC, 128, UN, 2, 128)
    s = np.stack([a, b], axis=3)                        # [cc, p, u, path, k, m]
    s = s.transpose(1, 2, 0, 3, 4, 5)                   # [p, u, cc, path, k, m]
    return np.ascontiguousarray(s.reshape(128, UN * UW)
                                ).astype(ml_dtypes.bfloat16)


def pack_wo(w):
    """wo [H, C] f32 -> [128, CB*BLK2] bf16 in the wob DRAM layout."""
    a = np.asarray(w).reshape(HH, 128, CB, CBW)          # [hh, p, cb, f]
    a = a.transpose(1, 2, 0, 3)                          # [p, cb, hh, f]
    return np.ascontiguousarray(a.reshape(128, CB * BLK2)
                                ).astype(ml_dtypes.bfloat16)


def pack_x(x_disp_T, tiles):
    """x^T dispatch slab [C, cap] f32 -> [128, CC*cap] bf16, tile-major
    per the given (t0, tw) tile list."""
    a = x_disp_T.reshape(CC, 128, x_disp_T.shape[1])        # [cc, p, t]
    parts = []
    for t0, tw in tiles:
        blk = a[:, :, t0:t0 + tw].transpose(1, 0, 2)        # [p, cc, tw]
        parts.append(blk.reshape(128, CC * tw))
    return np.ascontiguousarray(np.concatenate(parts, axis=1)
                                ).astype(ml_dtypes.bfloat16)


def _route(x, w_gate):
    """Host-side router. Runs the exact reference ops on jax-CPU so the
    top-2 selection and gate values match the reference bit-for-bit."""
    import jax
    import jax.numpy as jnp
    cpu = jax.devices("cpu")[0]
    with jax.default_device(cpu):
        xj = jnp.asarray(np.asarray(x))
        wj = jnp.asarray(np.asarray(w_gate))
        logits = jnp.einsum("btc,ce->bte", xj, wj)
        gates = jax.nn.softmax(logits, axis=-1)
        topk_vals, topk_idx = jax.lax.top_k(gates, TOP_K)
    return (np.asarray(topk_vals).reshape(-1, TOP_K),
            np.asarray(topk_idx).reshape(-1, TOP_K))


OV_W = 0          # overflow slot width of the last make_in_maps call
OV_PLAN = None    # per-core (src_expert, start_in_expert, count)


def _plan_overflow(loads, capm):
    """Split each expert's surplus beyond capm into chunks of one width
    W that fit the 8 per-core overflow slots. Returns (W, plan) where
    plan[c] = (src_expert, start_in_expert, count)."""
    ovf = [(e, n - capm) for e, n in enumerate(loads) if n > capm]
    if not ovf:
        return 0, [(c, 0, 0) for c in range(N_CORES)]
    W = 0
    for Wc in range(4, 4096, 4):
        if sum(-(-o // Wc) for _, o in ovf) <= N_CORES:
            W = Wc
            break
    assert W, f"overflow does not fit: {loads}"
    plan = []
    for e, o in ovf:
        s = 0
        while s < o:
            cnt = min(W, o - s)
            plan.append((e, capm + s, cnt))
            s += cnt
    while len(plan) < N_CORES:
        plan.append((len(plan) % N_EXPERTS, 0, 0))
    return W, plan


def _dispatch(x, topk_idx):
    """Token lists per expert, (token, slot) positions, cap, and the
    gathered+packed per-expert xtb slabs (+ overflow slabs when balancing)."""
    global OV_W, OV_PLAN
    N = x.shape[0] * x.shape[1] if x.ndim == 3 else x.shape[0]
    x_flat = np.ascontiguousarray(np.asarray(x).reshape(N, C))
    idx_lists = []
    pos = np.empty((N, TOP_K), dtype=np.int64)
    for e in range(N_EXPERTS):
        sel = (topk_idx == e)
        toks = np.flatnonzero(sel.any(axis=1))
        idx_lists.append(toks)
        pos_of = np.full(N, -1, dtype=np.int64)
        pos_of[toks] = np.arange(len(toks))
        for k in range(TOP_K):
            m = sel[:, k]
            pos[m, k] = pos_of[m]
    loads = [len(t) for t in idx_lists]
    max_cnt = max(loads)
    if BALANCE and max_cnt > 2048:
        # pick (cap_main, W) minimizing total columns cap_main + W
        best = None
        for capm in range(2048, 1916, -4):
            W, _ = _plan_overflow(loads, capm)
            if best is None or capm + W < best[0]:
                best = (capm + W, capm)
        cap = best[1]
        OV_W, OV_PLAN = _plan_overflow(loads, cap)
    else:
        cap = max(512, -(-max_cnt // 4) * 4)
        OV_W, OV_PLAN = 0, None
    tiles = tok_tiles_for(cap)

    xT = np.ascontiguousarray(x_flat.T)            # [C, N]
    xtbs, xobs = [], []
    for e in range(N_EXPERTS):
        toks = idx_lists[e]
        slab = np.zeros((C, cap), dtype=np.float32)
        n_main = min(len(toks), cap)
        slab[:, :n_main] = xT[:, toks[:n_main]]
        xtbs.append(pack_x(slab, tiles))
    if OV_W:
        for c in range(N_CORES):
            se, start, cnt = OV_PLAN[c]
            slab = np.zeros((C, OV_W), dtype=np.float32)
            if cnt:
                slab[:, :cnt] = xT[:, idx_lists[se][start:start + cnt]]
            xobs.append(pack_x(slab, [(0, OV_W)]))
    return idx_lists, pos, cap, (xtbs, xobs)


def make_in_maps(x, wi, wg, wo, topk_idx):
    idx_lists, pos, cap, (xtbs, xobs) = _dispatch(x, topk_idx)
    w1_packed = [None] * N_EXPERTS
    wo_packed = [None] * N_EXPERTS

    def w1p(e):
        if w1_packed[e] is None:
            w1_packed[e] = pack_w1(wi[e], wg[e])
        return w1_packed[e]

    def wop(e):
        if wo_packed[e] is None:
            wo_packed[e] = pack_wo(wo[e])
        return wo_packed[e]

    in_maps = []
    for e in range(N_EXPERTS):
        m = {"xtb": xtbs[e], "w1b": w1p(e), "wob": wop(e)}
        if OV_W:
            se = OV_PLAN[e][0]
            m["xob"] = xobs[e]
            m["w1o"] = w1p(se)
            m["wobo"] = wop(se)
        in_maps.append(m)
    return idx_lists, pos, cap, in_maps


def kernel(x, w_gate, wi, wg, wo):
    x = np.asarray(x)
    wi, wg, wo = np.asarray(wi), np.asarray(wg), np.asarray(wo)
    N = B * T

    topk_vals, topk_idx = _route(x, w_gate)
    idx_lists, pos, cap, in_maps = make_in_maps(x, wi, wg, wo, topk_idx)

    nc = build_program(cap, w_ov=OV_W)
    res = run_bass_kernel_spmd(nc, in_maps, core_ids=list(range(N_CORES)))

    # combine: out[t] = sum_k vals[t,k] * y_{idx[t,k]}[t]
    cap_full = max(cap, max(len(t) for t in idx_lists))
    Y = np.zeros((N_EXPERTS, cap_full, C), dtype=np.float32)  # token-major
    for e in range(N_EXPERTS):
        Y[e, :cap] = res.results[e]["yt"].astype(np.float32).T
    if OV_W:
        for c in range(N_CORES):
            se, start, cnt = OV_PLAN[c]
            if cnt:
                Y[se, start:start + cnt] = \
                    res.results[c]["yto"].astype(np.float32).T[:cnt]
    out = (topk_vals[:, 0:1] * Y[topk_idx[:, 0], pos[:, 0], :]
           + topk_vals[:, 1:2] * Y[topk_idx[:, 1], pos[:, 1], :])
    return out.reshape(B, T, C).astype(np.float32)
